# revision 5
# baseline (speedup 1.0000x reference)
"""DenseCorr2d full kernel for 8 Trainium2 NeuronCores.

Reference computation (per example b):
  corr[(cm*16+ct), y, x] = sum_{dy,dx} tm_edgepad[cm, y+dy, x+dx] * tp[ct, dy, dx]
  out[co, y, x] = bias[co] + sum_{ci,ky,kx} W[co, ci, ky, kx] * corr_zpad[ci, y+ky-1, x+kx-1]

Sharding: data-parallel over batch; core i computes example i entirely.

Stage A (dense correlation) folds the template taps into the matmul
contraction dim: with y = 8w + j and dx = 4a + de, the contraction rows are
(f = j+dy, de) = 92 partitions, the stationary columns are (ct, j) = 128
(fully dense), and accumulation over a happens in PSUM (4 matmuls per
128-row x 512-spatial tile).  The moving operand baseT[cm, (f,de), w, x'] =
tm_pad[cm, 8w+f, x'+de] is precomputed on host so each tile load is a
contiguous-per-partition DMA.  PSUM is evacuated through an SBUF staging
tile (cast fp32->bf16) and a per-cm DMA de-interleaves j into the standard
corr row order.

Stage B runs the 3x3 'same' merge conv as 18 PSUM-accumulating matmuls
(9 taps x 2 input-channel chunks) per 3-row band with the tap shift
expressed as a free-dim offset into the zero-padded corr; bias is fused
into the ScalarE PSUM->SBUF copy.

All matmuls run in bf16 (inputs are unit-normal; accumulation in fp32
PSUM keeps the relative error ~5e-3, well inside the 2e-2 gate).
"""

from contextlib import ExitStack

import ml_dtypes
import numpy as np

import concourse.bass as bass
import concourse.tile as tile
from concourse import bacc, mybir
from concourse.bass_utils import run_bass_kernel_spmd

F32 = mybir.dt.float32
BF16 = mybir.dt.bfloat16

N_CORES = 8
# Problem shapes (hardcoded per contract).
B, CT, HT, WT = 8, 16, 16, 16
CM, HM, WM = 16, 128, 128
COUT, K = 64, 3
HP = HM + HT - 1  # 143 padded image rows/cols
NF = 23  # f = j + dy range
NP = 4 * NF  # 92 contraction rows (f, de)
XW = 140  # x' range of baseT (x + 4a <= 127+12)

_CACHE: dict = {}


def _emit(ctx: ExitStack, tc, nc, btT, sa2, wst, bia, zzb, out):
    const = ctx.enter_context(tc.tile_pool(name="const", bufs=1))
    corrp = ctx.enter_context(tc.tile_pool(name="corrp", bufs=1))

    sa2_sb = const.tile([NP, 4, 128], BF16, name="sa2_sb")
    nc.sync.dma_start(out=sa2_sb[:], in_=sa2.ap())
    w_sb = const.tile([128, 18, COUT], BF16, name="w_sb")
    nc.scalar.dma_start(out=w_sb[:, :9], in_=wst.ap()[:, :9])
    nc.scalar.dma_start(out=w_sb[:, 9:], in_=wst.ap()[:, 9:])
    b_sb = const.tile([COUT, 1], F32, name="b_sb")
    nc.scalar.dma_start(out=b_sb[:], in_=bia.ap())

    # corr, zero-padded: 2 chunks of 130x130 rows/cols, chunk c = channels
    # [c*128, (c+1)*128) on partitions.
    # 2 elements of tail slack: the last band's kx-shifted windows read (and
    # discard) up to 2 elements past the padded grid.
    corr_sb = corrp.tile([128, 2 * 130 * 130 + 2], BF16, name="corr_sb")
    corr_flat = corr_sb[:]
    corr = corr_sb[:, : 2 * 130 * 130].rearrange("p (a b) -> p a b", a=2 * 130)
    # Zero the padding borders by DMA from a host-supplied zeros tensor.
    nc.scalar.dma_start(out=corr_sb[:, 2 * 130 * 130 :], in_=zzb.ap()[:, :2])
    for c in range(2):
        nc.scalar.dma_start(out=corr[:, c * 130, :], in_=zzb.ap()[:, :130])
        nc.scalar.dma_start(out=corr[:, c * 130 + 129, :], in_=zzb.ap()[:, :130])
        nc.scalar.dma_start(
            out=corr[:, c * 130 : (c + 1) * 130, 0], in_=zzb.ap()[:, :130]
        )
        nc.scalar.dma_start(
            out=corr[:, c * 130 : (c + 1) * 130, 129], in_=zzb.ap()[:, :130]
        )

    # ---- Stage A ----
    # Per cm-pair: load baseT tiles, run 4 PSUM-accumulated matmuls per
    # (cm, ybg) with the dx-group offset 4a as a free-dim offset, evacuate
    # through a staging tile, and DMA-shuffle j back into row order.
    with (
        tc.tile_pool(name="btp", bufs=2) as btp,
        tc.tile_pool(name="stp", bufs=2) as stp,
        tc.tile_pool(name="psA", bufs=8, space="PSUM") as psA,
    ):
        for pr in range(8):
            bts = []
            for cm2 in range(2):
                cm = 2 * pr + cm2
                bt = btp.tile([NP, 16, XW], BF16, name=f"bt{cm2}", tag=f"bt{cm2}")
                for k in range(4):
                    nc.sync.dma_start(
                        out=bt[23 * k : 23 * k + 23],
                        in_=btT.ap()[cm, 23 * k : 23 * k + 23],
                    )
                bts.append(bt)
            pts = [
                [
                    psA.tile([128, 4, 128], F32, name=f"pA{cm2}_{ybg}", tag="pA")
                    for ybg in range(4)
                ]
                for cm2 in range(2)
            ]
            for a in range(4):
                for cm2 in range(2):
                    for ybg in range(4):
                        nc.tensor.matmul(
                            pts[cm2][ybg][:],
                            sa2_sb[:, a, :],
                            bts[cm2][:, 4 * ybg : 4 * ybg + 4, 4 * a : 4 * a + 128],
                            start=(a == 0),
                            stop=(a == 3),
                        )
            for cm2 in range(2):
                cm = 2 * pr + cm2
                st = stp.tile([128, 16, 128], BF16, name=f"st{cm2}", tag=f"st{cm2}")
                for ybg in range(4):
                    dst_sl = st[:, 4 * ybg : 4 * ybg + 4, :]
                    if ybg % 2 == 0:
                        nc.vector.tensor_copy(dst_sl, pts[cm2][ybg][:])
                    else:
                        nc.scalar.copy(dst_sl, pts[cm2][ybg][:])
                # shuffle: st[(8ct+j), w, x] -> corr rows (8w+j), cols 1..129
                # (8 per-j DMAs: the AP balancer caps DMAs at 3 dims)
                c, cmh = cm // 8, cm % 8
                for j in range(8):
                    r0 = c * 130 + 1 + j
                    eng = nc.sync if (j % 2 == 0) else nc.scalar
                    eng.dma_start(
                        out=corr[
                            16 * cmh : 16 * cmh + 16, r0 : r0 + 122 : 8, 1:129
                        ],
                        in_=st[:][j::8],
                    )

    # ---- Stage B ----
    BAND = 3
    with (
        tc.tile_pool(name="psB", bufs=4, space="PSUM") as psB,
        tc.tile_pool(name="outp", bufs=2) as outp,
    ):
        n_bands = (HM + BAND - 1) // BAND
        GRP = 8  # bands per output DMA batch
        ot = None
        ot_base = 0
        for band in range(n_bands):
            if band % GRP == 0:
                if ot is not None:
                    g_rows = BAND * band - ot_base
                    nc.scalar.dma_start(
                        out=out.ap()[:, ot_base : ot_base + g_rows, :],
                        in_=ot[:, :g_rows, :],
                    )
                ot = outp.tile([COUT, GRP * BAND, WM], F32, name="ot", tag="ot")
                ot_base = BAND * band
            y0 = BAND * band
            rows = min(BAND, HM - y0)
            n = rows * 130
            pb = psB.tile([COUT, BAND * 130], F32, name="pb", tag="pb")
            for c in range(2):
                for s in range(9):
                    ky, kx = divmod(s, 3)
                    off = (c * 130 + y0 + ky) * 130 + kx
                    nc.tensor.matmul(
                        pb[:, :n],
                        w_sb[:, c * 9 + s, :],
                        corr_flat[:, off : off + n],
                        start=(c == 0 and s == 0),
                        stop=(c == 1 and s == 8),
                    )
            nc.scalar.activation(
                ot[:, y0 - ot_base : y0 - ot_base + rows, :],
                pb[:, : rows * 130].rearrange("p (a b) -> p a b", b=130)[
                    :, :, 0:128
                ],
                mybir.ActivationFunctionType.Identity,
                bias=b_sb[:, 0:1],
            )
        g_rows = HM - ot_base
        nc.scalar.dma_start(
            out=out.ap()[:, ot_base : ot_base + g_rows, :],
            in_=ot[:, :g_rows, :],
        )


def _build():
    nc = bacc.Bacc("TRN2", target_bir_lowering=False, debug=False)
    btT = nc.dram_tensor("btT", [CM, NP, 16, XW], BF16, kind="ExternalInput")
    sa2 = nc.dram_tensor("sa2", [NP, 4, 128], BF16, kind="ExternalInput")
    wst = nc.dram_tensor("wst", [128, 18, COUT], BF16, kind="ExternalInput")
    bia = nc.dram_tensor("bias", [COUT, 1], F32, kind="ExternalInput")
    zzb = nc.dram_tensor("zzb", [128, 130], BF16, kind="ExternalInput")
    out = nc.dram_tensor("out", [COUT, HM, WM], F32, kind="ExternalOutput")
    with tile.TileContext(nc) as tc, ExitStack() as ctx:
        _emit(ctx, tc, nc, btT, sa2, wst, bia, zzb, out)
    nc.compile()
    return nc


def _get_nc():
    if "nc" not in _CACHE:
        _CACHE["nc"] = _build()
    return _CACHE["nc"]


def _host_prep(template, tomatch, W, b):
    template = np.ascontiguousarray(template, dtype=np.float32)
    tomatch = np.ascontiguousarray(tomatch, dtype=np.float32)
    W = np.ascontiguousarray(W, dtype=np.float32)
    b = np.ascontiguousarray(b, dtype=np.float32)
    bf = ml_dtypes.bfloat16

    tm_pad = np.pad(
        tomatch, ((0, 0), (0, 0), (0, HT - 1), (0, WT - 1)), mode="edge"
    )  # [B, CM, 143, 143]

    # baseT[b, cm, 4f+de, w, x'] = tm_pad[b, cm, 8w+f, x'+de]
    s0, s1, s2, s3 = tm_pad.strides
    bview = np.lib.stride_tricks.as_strided(
        tm_pad,
        shape=(B, CM, NF, 4, 16, XW),
        strides=(s0, s1, s2, s3, 8 * s2, s3),
    )
    btT = np.ascontiguousarray(bview).reshape(B, CM, NP, 16, XW).astype(bf)

    # sa2[b, 4f+de, a, 8ct+j] = template[b, ct, f-j, 4a+de] for 0<=f-j<16
    sa2 = np.zeros((B, NP, 4, 128), np.float32)
    tview = template.reshape(B, CT, HT, 4, 4)  # [b, ct, dy, a, de]
    for j in range(8):
        for dy in range(HT):
            f = j + dy
            # [b, de, a, ct] slab
            sa2[:, 4 * f : 4 * f + 4, :, j::8] = tview[:, :, dy].transpose(
                0, 3, 2, 1
            )
    sa2 = sa2.astype(bf)

    # wst[k, c*9 + ky*3 + kx, co] = W[co, c*128+k, ky, kx]
    wst = np.ascontiguousarray(
        W.reshape(COUT, 2, 128, K, K).transpose(2, 1, 3, 4, 0).reshape(128, 18, COUT)
    ).astype(bf)
    bias = np.ascontiguousarray(b.reshape(COUT, 1))
    zzb = np.zeros((128, 130), bf)
    return btT, sa2, wst, bias, zzb


def _in_maps(template, tomatch, W, b):
    btT, sa2, wst, bias, zzb = _host_prep(template, tomatch, W, b)
    return [
        {"btT": btT[i], "sa2": sa2[i], "wst": wst, "bias": bias, "zzb": zzb}
        for i in range(N_CORES)
    ]


def kernel(template, tomatch, W, b):
    in_maps = _in_maps(template, tomatch, W, b)
    nc = _get_nc()
    res = run_bass_kernel_spmd(nc, in_maps, list(range(N_CORES)))
    return np.stack([res.results[i]["out"] for i in range(N_CORES)])


# revision 6
# speedup vs baseline: 1.2216x; 1.2216x over previous
"""DenseCorr2d full kernel for 8 Trainium2 NeuronCores.

Reference computation (per example b):
  corr[(cm*16+ct), y, x] = sum_{dy,dx} tm_edgepad[cm, y+dy, x+dx] * tp[ct, dy, dx]
  out[co, y, x] = bias[co] + sum_{ci,ky,kx} W[co, ci, ky, kx] * corr_zpad[ci, y+ky-1, x+kx-1]

Sharding: data-parallel over batch; core i computes example i entirely.

Stage A (dense correlation) folds the template taps into the matmul
contraction dim: with y = 8w + j and dx = 4a + de, the contraction rows are
(f = j+dy, de) = 92 partitions, the stationary columns are (ct, j) = 128
(fully dense), and accumulation over a happens in PSUM (4 matmuls per
128-col x 512-spatial tile).  The moving operand baseT[cm, (f,de), w, x'] =
tm_pad[cm, 8w+f, x'+de] is precomputed on host so each tile load is a
contiguous-per-partition DMA.

corr lives in SBUF j-interleaved: corr[ci, (c, j, ws, x)] holds row
y = 8*(ws-1) + j of chunk c (ws=0/17 are the y=-1/128 zero-pad rows, x is
130 wide with zero x-borders).  This makes the PSUM evacuation a full-rate
copy (cast fp32->bf16 into a staging tile) plus ONE contiguous-run DMA per
cm, and keeps every stage-B tap window a contiguous slice within a slab.

Stage B runs the 3x3 'same' merge conv over residue bands (fixed j0, a
window of w): 18 PSUM-accumulating matmuls (9 taps x 2 input-channel
chunks) per band, tap shifts being flat offsets into the slab; bias fused
into the ScalarE PSUM->SBUF copy, which scatters rows back into the
row-major output tile.

All matmuls run in bf16 (inputs are unit-normal; accumulation in fp32
PSUM keeps the relative error ~4e-3, well inside the 2e-2 gate).
"""

from contextlib import ExitStack

import ml_dtypes
import numpy as np

import concourse.bass as bass
import concourse.tile as tile
from concourse import bacc, mybir
from concourse.bass_utils import run_bass_kernel_spmd

F32 = mybir.dt.float32
BF16 = mybir.dt.bfloat16

N_CORES = 8
# Problem shapes (hardcoded per contract).
B, CT, HT, WT = 8, 16, 16, 16
CM, HM, WM = 16, 128, 128
COUT, K = 64, 3
HP = HM + HT - 1  # 143 padded image rows/cols
NF = 23  # f = j + dy range
NP = 4 * NF  # 92 contraction rows (f, de)
XW = 140  # x' range of baseT (x + 4a <= 127+12)
SLAB = 18 * 130  # corr slab (c, j): 18 ws-rows of 130
NCORR = 16 * SLAB  # 2 chunks * 8 j

# stage-B w-window split: 16 w values per j0-residue, psum <= 512 fp32
RWS = [3, 3, 3, 3, 2, 2]
W0S = [0, 3, 6, 9, 12, 14]

_CACHE: dict = {}


def _emit(ctx: ExitStack, tc, nc, btT, sa2, wst, bia, zzb, out):
    const = ctx.enter_context(tc.tile_pool(name="const", bufs=1))
    corrp = ctx.enter_context(tc.tile_pool(name="corrp", bufs=1))

    sa2_sb = const.tile([NP, 4, 128], BF16, name="sa2_sb")
    nc.sync.dma_start(out=sa2_sb[:], in_=sa2.ap())
    w_sb = const.tile([128, 18, COUT], BF16, name="w_sb")
    nc.scalar.dma_start(out=w_sb[:, :9], in_=wst.ap()[:, :9])
    nc.scalar.dma_start(out=w_sb[:, 9:], in_=wst.ap()[:, 9:])
    b_sb = const.tile([COUT, 1], F32, name="b_sb")
    nc.scalar.dma_start(out=b_sb[:], in_=bia.ap())

    corr_sb = corrp.tile([128, NCORR + 2], BF16, name="corr_sb")
    corr_flat = corr_sb[:]
    # slab view: [p, c*8+j, ws*130+x]
    corr_j = corr_sb[:, :NCORR].rearrange("p (s t) -> p s t", s=16)
    # tail slack + the only zero rows stage B ever reads:
    # (c, j=7, ws=0) = row y=-1 and (c, j=0, ws=17) = row y=128.
    nc.scalar.dma_start(out=corr_sb[:, NCORR:], in_=zzb.ap()[:, :2])
    for c in range(2):
        nc.scalar.dma_start(
            out=corr_j[:, c * 8 + 7, 0:130], in_=zzb.ap()[:, :130]
        )
        nc.scalar.dma_start(
            out=corr_j[:, c * 8, 17 * 130 : 18 * 130], in_=zzb.ap()[:, :130]
        )

    # ---- Stage A ----
    with (
        tc.tile_pool(name="btp0", bufs=2) as btp0,
        tc.tile_pool(name="btp1", bufs=2) as btp1,
        tc.tile_pool(name="stp0", bufs=2) as stp0,
        tc.tile_pool(name="stp1", bufs=2) as stp1,
        tc.tile_pool(name="psA", bufs=8, space="PSUM") as psA,
    ):
        btps, stps = [btp0, btp1], [stp0, stp1]
        for pr in range(8):
            bts = []
            for cm2 in range(2):
                cm = 2 * pr + cm2
                bt = btps[cm2].tile([NP, 16, XW], BF16, name="bt", tag="bt")
                for k in range(4):
                    nc.sync.dma_start(
                        out=bt[23 * k : 23 * k + 23],
                        in_=btT.ap()[cm, 23 * k : 23 * k + 23],
                    )
                bts.append(bt)
            pts = [
                [
                    psA.tile([128, 4, 128], F32, name=f"pA{cm2}_{ybg}", tag="pA")
                    for ybg in range(4)
                ]
                for cm2 in range(2)
            ]
            for a in range(4):
                for cm2 in range(2):
                    for ybg in range(4):
                        nc.tensor.matmul(
                            pts[cm2][ybg][:],
                            sa2_sb[:, a, :],
                            bts[cm2][:, 4 * ybg : 4 * ybg + 4, 4 * a : 4 * a + 128],
                            start=(a == 0),
                            stop=(a == 3),
                        )
            for cm2 in range(2):
                cm = 2 * pr + cm2
                st = stps[cm2].tile([128, 16, 130], BF16, name="st", tag="st")
                # zero x-borders; copies fill cols 1..129
                nc.gpsimd.memset(st[:, :, 0], 0.0)
                nc.gpsimd.memset(st[:, :, 129], 0.0)
                for ybg in range(4):
                    dst_sl = st[:, 4 * ybg : 4 * ybg + 4, 1:129]
                    if ybg % 2 == 0:
                        nc.vector.tensor_copy(dst_sl, pts[cm2][ybg][:])
                    else:
                        nc.scalar.copy(dst_sl, pts[cm2][ybg][:])
                # st[(8ct+j), w, x] -> corr slabs (c, j), ws = w+1
                c, cmh = cm // 8, cm % 8
                eng = nc.sync if cm2 == 0 else nc.scalar
                eng.dma_start(
                    out=corr_j[
                        16 * cmh : 16 * cmh + 16, c * 8 : c * 8 + 8, 130 : 130 + 2080
                    ],
                    in_=st[:],
                )

    # ---- Stage B ----
    def slab_off(c, j0, ky, w0):
        jj = j0 + ky - 1
        if jj < 0:
            return (c * 8 + 7) * SLAB + w0 * 130
        if jj > 7:
            return c * 8 * SLAB + (w0 + 2) * 130
        return (c * 8 + jj) * SLAB + (w0 + 1) * 130

    with (
        tc.tile_pool(name="psB", bufs=4, space="PSUM") as psB,
        tc.tile_pool(name="outp", bufs=2) as outp,
    ):
        for wg in range(len(RWS)):
            w0, rw = W0S[wg], RWS[wg]
            ot = outp.tile([COUT, 8 * rw, WM], F32, name="ot", tag="ot")
            for j0 in range(8):
                n = rw * 130
                pb = psB.tile([COUT, 3 * 130], F32, name="pb", tag="pb")
                first = True
                for c in range(2):
                    for s in range(9):
                        ky, kx = divmod(s, 3)
                        off = slab_off(c, j0, ky, w0) + kx
                        nc.tensor.matmul(
                            pb[:, :n],
                            w_sb[:, c * 9 + s, :],
                            corr_flat[:, off : off + n],
                            start=first,
                            stop=(c == 1 and s == 8),
                        )
                        first = False
                nc.scalar.activation(
                    ot[:, j0 : j0 + 8 * (rw - 1) + 1 : 8, :],
                    pb[:, :n].rearrange("p (a b) -> p a b", b=130)[:, :, 0:128],
                    mybir.ActivationFunctionType.Identity,
                    bias=b_sb[:, 0:1],
                )
            nc.scalar.dma_start(
                out=out.ap()[:, 8 * w0 : 8 * w0 + 8 * rw, :],
                in_=ot[:],
            )


def _build():
    nc = bacc.Bacc("TRN2", target_bir_lowering=False, debug=False)
    btT = nc.dram_tensor("btT", [CM, NP, 16, XW], BF16, kind="ExternalInput")
    sa2 = nc.dram_tensor("sa2", [NP, 4, 128], BF16, kind="ExternalInput")
    wst = nc.dram_tensor("wst", [128, 18, COUT], BF16, kind="ExternalInput")
    bia = nc.dram_tensor("bias", [COUT, 1], F32, kind="ExternalInput")
    zzb = nc.dram_tensor("zzb", [128, 130], BF16, kind="ExternalInput")
    out = nc.dram_tensor("out", [COUT, HM, WM], F32, kind="ExternalOutput")
    with tile.TileContext(nc) as tc, ExitStack() as ctx:
        _emit(ctx, tc, nc, btT, sa2, wst, bia, zzb, out)
    nc.compile()
    return nc


def _get_nc():
    if "nc" not in _CACHE:
        _CACHE["nc"] = _build()
    return _CACHE["nc"]


def _host_prep(template, tomatch, W, b):
    template = np.ascontiguousarray(template, dtype=np.float32)
    tomatch = np.ascontiguousarray(tomatch, dtype=np.float32)
    W = np.ascontiguousarray(W, dtype=np.float32)
    b = np.ascontiguousarray(b, dtype=np.float32)
    bf = ml_dtypes.bfloat16

    tm_pad = np.pad(
        tomatch, ((0, 0), (0, 0), (0, HT - 1), (0, WT - 1)), mode="edge"
    )  # [B, CM, 143, 143]

    # baseT[b, cm, 4f+de, w, x'] = tm_pad[b, cm, 8w+f, x'+de]
    s0, s1, s2, s3 = tm_pad.strides
    bview = np.lib.stride_tricks.as_strided(
        tm_pad,
        shape=(B, CM, NF, 4, 16, XW),
        strides=(s0, s1, s2, s3, 8 * s2, s3),
    )
    btT = np.ascontiguousarray(bview).reshape(B, CM, NP, 16, XW).astype(bf)

    # sa2[b, 4f+de, a, 8ct+j] = template[b, ct, f-j, 4a+de] for 0<=f-j<16
    sa2 = np.zeros((B, NP, 4, 128), np.float32)
    tview = template.reshape(B, CT, HT, 4, 4)  # [b, ct, dy, a, de]
    for j in range(8):
        for dy in range(HT):
            f = j + dy
            # [b, de, a, ct] slab
            sa2[:, 4 * f : 4 * f + 4, :, j::8] = tview[:, :, dy].transpose(
                0, 3, 2, 1
            )
    sa2 = sa2.astype(bf)

    # wst[k, c*9 + ky*3 + kx, co] = W[co, c*128+k, ky, kx]
    wst = np.ascontiguousarray(
        W.reshape(COUT, 2, 128, K, K).transpose(2, 1, 3, 4, 0).reshape(128, 18, COUT)
    ).astype(bf)
    bias = np.ascontiguousarray(b.reshape(COUT, 1))
    zzb = np.zeros((128, 130), bf)
    return btT, sa2, wst, bias, zzb


def _in_maps(template, tomatch, W, b):
    btT, sa2, wst, bias, zzb = _host_prep(template, tomatch, W, b)
    return [
        {"btT": btT[i], "sa2": sa2[i], "wst": wst, "bias": bias, "zzb": zzb}
        for i in range(N_CORES)
    ]


def kernel(template, tomatch, W, b):
    in_maps = _in_maps(template, tomatch, W, b)
    nc = _get_nc()
    res = run_bass_kernel_spmd(nc, in_maps, list(range(N_CORES)))
    return np.stack([res.results[i]["out"] for i in range(N_CORES)])


# revision 7
# speedup vs baseline: 1.6882x; 1.3819x over previous
"""DenseCorr2d full kernel for 8 Trainium2 NeuronCores.

Reference computation (per example b):
  corr[(cm*16+ct), y, x] = sum_{dy,dx} tm_edgepad[cm, y+dy, x+dx] * tp[ct, dy, dx]
  out[co, y, x] = bias[co] + sum_{ci,ky,kx} W[co, ci, ky, kx] * corr_zpad[ci, y+ky-1, x+kx-1]

Sharding: data-parallel over batch; core i computes example i entirely.

Stage A (dense correlation) folds the template taps into the matmul
contraction dim: with y = 8w + j and dx = 4a + de, the contraction rows are
(f = j+dy, de) = 92 partitions, the stationary columns are (ct, j) = 128
(fully dense), and accumulation over a happens in PSUM (4 matmuls per
128-col x 512-spatial tile).  The moving operand baseT[cm, (f,de), w, x'] =
tm_pad[cm, 8w+f, x'+de] is precomputed on host so each tile load is a
contiguous-per-partition DMA.

corr lives in SBUF j-interleaved: corr[ci, (c, j, ws, x)] holds row
y = 8*(ws-1) + j of chunk c (ws=0/17 are the y=-1/128 zero-pad rows, x is
130 wide with zero x-borders).  This makes the PSUM evacuation a full-rate
copy (cast fp32->bf16 into a staging tile) plus ONE contiguous-run DMA per
cm, and keeps every stage-B tap window a contiguous slice within a slab.

Stage B runs the 3x3 'same' merge conv over residue bands (fixed j0, a
window of w): 18 PSUM-accumulating matmuls (9 taps x 2 input-channel
chunks) per band, tap shifts being flat offsets into the slab; bias fused
into the ScalarE PSUM->SBUF copy, which scatters rows back into the
row-major output tile.

All matmuls run in bf16 (inputs are unit-normal; accumulation in fp32
PSUM keeps the relative error ~4e-3, well inside the 2e-2 gate).
"""

from contextlib import ExitStack

import ml_dtypes
import numpy as np

import concourse.bass as bass
import concourse.tile as tile
from concourse import bacc, mybir
from concourse.bass_utils import run_bass_kernel_spmd

F32 = mybir.dt.float32
BF16 = mybir.dt.bfloat16

N_CORES = 8
# Problem shapes (hardcoded per contract).
B, CT, HT, WT = 8, 16, 16, 16
CM, HM, WM = 16, 128, 128
COUT, K = 64, 3
HP = HM + HT - 1  # 143 padded image rows/cols
NF = 23  # f = j + dy range
NP = 4 * NF  # 92 contraction rows (f, de)
XW = 140  # x' range of baseT (x + 4a <= 127+12)
SLAB = 18 * 130  # corr slab (c, j): 18 ws-rows of 130
NCORR = 16 * SLAB  # 2 chunks * 8 j

# stage-B w-window split: 16 w values per j0-residue, psum <= 512 fp32
RWS = [3, 3, 3, 3, 2, 2]
W0S = [0, 3, 6, 9, 12, 14]

_CACHE: dict = {}


def _emit(ctx: ExitStack, tc, nc, btT, sa2, wst, bia, zzb, out):
    const = ctx.enter_context(tc.tile_pool(name="const", bufs=1))
    corrp = ctx.enter_context(tc.tile_pool(name="corrp", bufs=1))

    sa2_sb = const.tile([NP, 4, 128], BF16, name="sa2_sb")
    nc.sync.dma_start(out=sa2_sb[:], in_=sa2.ap())
    w_sb = const.tile([128, 18, COUT], BF16, name="w_sb")
    nc.scalar.dma_start(out=w_sb[:, :9], in_=wst.ap()[:, :9])
    nc.scalar.dma_start(out=w_sb[:, 9:], in_=wst.ap()[:, 9:])
    b_sb = const.tile([COUT, 1], F32, name="b_sb")
    nc.scalar.dma_start(out=b_sb[:], in_=bia.ap())

    corr_sb = corrp.tile([128, NCORR + 2], BF16, name="corr_sb")
    corr_flat = corr_sb[:]
    # slab view: [p, c*8+j, ws*130+x]
    corr_j = corr_sb[:, :NCORR].rearrange("p (s t) -> p s t", s=16)
    # tail slack + the only zero rows stage B ever reads:
    # (c, j=7, ws=0) = row y=-1 and (c, j=0, ws=17) = row y=128.
    nc.scalar.dma_start(out=corr_sb[:, NCORR:], in_=zzb.ap()[:, :2])
    for c in range(2):
        nc.scalar.dma_start(
            out=corr_j[:, c * 8 + 7, 0:130], in_=zzb.ap()[:, :130]
        )
        nc.scalar.dma_start(
            out=corr_j[:, c * 8, 17 * 130 : 18 * 130], in_=zzb.ap()[:, :130]
        )

    # ---- Stage A ----
    with (
        tc.tile_pool(name="btp0", bufs=2) as btp0,
        tc.tile_pool(name="btp1", bufs=2) as btp1,
        tc.tile_pool(name="stp0", bufs=2) as stp0,
        tc.tile_pool(name="stp1", bufs=2) as stp1,
        tc.tile_pool(name="psA", bufs=8, space="PSUM") as psA,
    ):
        btps, stps = [btp0, btp1], [stp0, stp1]
        for pr in range(8):
            bts = []
            for cm2 in range(2):
                cm = 2 * pr + cm2
                bt = btps[cm2].tile([NP, 16, XW], BF16, name="bt", tag="bt")
                # one big dma_start: the HW splits an InstDMACopy across
                # SDMA engines at packet granularity (~12 descriptors), so
                # 92 partition-descriptors spread over ~8 engines.
                nc.sync.dma_start(out=bt[:], in_=btT.ap()[cm])
                bts.append(bt)
            pts = [
                [
                    psA.tile([128, 4, 128], F32, name=f"pA{cm2}_{ybg}", tag="pA")
                    for ybg in range(4)
                ]
                for cm2 in range(2)
            ]
            for a in range(4):
                for cm2 in range(2):
                    for ybg in range(4):
                        nc.tensor.matmul(
                            pts[cm2][ybg][:],
                            sa2_sb[:, a, :],
                            bts[cm2][:, 4 * ybg : 4 * ybg + 4, 4 * a : 4 * a + 128],
                            start=(a == 0),
                            stop=(a == 3),
                        )
            for cm2 in range(2):
                cm = 2 * pr + cm2
                st = stps[cm2].tile([128, 16, 130], BF16, name="st", tag="st")
                # zero x-borders; copies fill cols 1..129
                nc.gpsimd.memset(st[:, :, 0], 0.0)
                nc.gpsimd.memset(st[:, :, 129], 0.0)
                for ybg in range(4):
                    dst_sl = st[:, 4 * ybg : 4 * ybg + 4, 1:129]
                    if ybg % 2 == 0:
                        nc.vector.tensor_copy(dst_sl, pts[cm2][ybg][:])
                    else:
                        nc.scalar.copy(dst_sl, pts[cm2][ybg][:])
                # st[(8ct+j), w, x] -> corr slabs (c, j), ws = w+1
                c, cmh = cm // 8, cm % 8
                eng = nc.sync if cm2 == 0 else nc.scalar
                eng.dma_start(
                    out=corr_j[
                        16 * cmh : 16 * cmh + 16, c * 8 : c * 8 + 8, 130 : 130 + 2080
                    ],
                    in_=st[:],
                )

    # ---- Stage B ----
    def slab_off(c, j0, ky, w0):
        jj = j0 + ky - 1
        if jj < 0:
            return (c * 8 + 7) * SLAB + w0 * 130
        if jj > 7:
            return c * 8 * SLAB + (w0 + 2) * 130
        return (c * 8 + jj) * SLAB + (w0 + 1) * 130

    with (
        tc.tile_pool(name="psB", bufs=4, space="PSUM") as psB,
        tc.tile_pool(name="outp", bufs=2) as outp,
    ):
        for wg in range(len(RWS)):
            w0, rw = W0S[wg], RWS[wg]
            ot = outp.tile([COUT, 8 * rw, WM], F32, name="ot", tag="ot")
            for j0 in range(8):
                n = rw * 130
                pb = psB.tile([COUT, 3 * 130], F32, name="pb", tag="pb")
                first = True
                for c in range(2):
                    for s in range(9):
                        ky, kx = divmod(s, 3)
                        off = slab_off(c, j0, ky, w0) + kx
                        nc.tensor.matmul(
                            pb[:, :n],
                            w_sb[:, c * 9 + s, :],
                            corr_flat[:, off : off + n],
                            start=first,
                            stop=(c == 1 and s == 8),
                        )
                        first = False
                nc.scalar.activation(
                    ot[:, j0 : j0 + 8 * (rw - 1) + 1 : 8, :],
                    pb[:, :n].rearrange("p (a b) -> p a b", b=130)[:, :, 0:128],
                    mybir.ActivationFunctionType.Identity,
                    bias=b_sb[:, 0:1],
                )
            nc.scalar.dma_start(
                out=out.ap()[:, 8 * w0 : 8 * w0 + 8 * rw, :],
                in_=ot[:],
            )


def _build():
    nc = bacc.Bacc("TRN2", target_bir_lowering=False, debug=False)
    btT = nc.dram_tensor("btT", [CM, NP, 16, XW], BF16, kind="ExternalInput")
    sa2 = nc.dram_tensor("sa2", [NP, 4, 128], BF16, kind="ExternalInput")
    wst = nc.dram_tensor("wst", [128, 18, COUT], BF16, kind="ExternalInput")
    bia = nc.dram_tensor("bias", [COUT, 1], F32, kind="ExternalInput")
    zzb = nc.dram_tensor("zzb", [128, 130], BF16, kind="ExternalInput")
    out = nc.dram_tensor("out", [COUT, HM, WM], F32, kind="ExternalOutput")
    with tile.TileContext(nc) as tc, ExitStack() as ctx:
        _emit(ctx, tc, nc, btT, sa2, wst, bia, zzb, out)
    nc.compile()
    return nc


def _get_nc():
    if "nc" not in _CACHE:
        _CACHE["nc"] = _build()
    return _CACHE["nc"]


def _host_prep(template, tomatch, W, b):
    template = np.ascontiguousarray(template, dtype=np.float32)
    tomatch = np.ascontiguousarray(tomatch, dtype=np.float32)
    W = np.ascontiguousarray(W, dtype=np.float32)
    b = np.ascontiguousarray(b, dtype=np.float32)
    bf = ml_dtypes.bfloat16

    tm_pad = np.pad(
        tomatch, ((0, 0), (0, 0), (0, HT - 1), (0, WT - 1)), mode="edge"
    )  # [B, CM, 143, 143]

    # baseT[b, cm, 4f+de, w, x'] = tm_pad[b, cm, 8w+f, x'+de]
    s0, s1, s2, s3 = tm_pad.strides
    bview = np.lib.stride_tricks.as_strided(
        tm_pad,
        shape=(B, CM, NF, 4, 16, XW),
        strides=(s0, s1, s2, s3, 8 * s2, s3),
    )
    btT = np.ascontiguousarray(bview).reshape(B, CM, NP, 16, XW).astype(bf)

    # sa2[b, 4f+de, a, 8ct+j] = template[b, ct, f-j, 4a+de] for 0<=f-j<16
    sa2 = np.zeros((B, NP, 4, 128), np.float32)
    tview = template.reshape(B, CT, HT, 4, 4)  # [b, ct, dy, a, de]
    for j in range(8):
        for dy in range(HT):
            f = j + dy
            # [b, de, a, ct] slab
            sa2[:, 4 * f : 4 * f + 4, :, j::8] = tview[:, :, dy].transpose(
                0, 3, 2, 1
            )
    sa2 = sa2.astype(bf)

    # wst[k, c*9 + ky*3 + kx, co] = W[co, c*128+k, ky, kx]
    wst = np.ascontiguousarray(
        W.reshape(COUT, 2, 128, K, K).transpose(2, 1, 3, 4, 0).reshape(128, 18, COUT)
    ).astype(bf)
    bias = np.ascontiguousarray(b.reshape(COUT, 1))
    zzb = np.zeros((128, 130), bf)
    return btT, sa2, wst, bias, zzb


def _in_maps(template, tomatch, W, b):
    btT, sa2, wst, bias, zzb = _host_prep(template, tomatch, W, b)
    return [
        {"btT": btT[i], "sa2": sa2[i], "wst": wst, "bias": bias, "zzb": zzb}
        for i in range(N_CORES)
    ]


def kernel(template, tomatch, W, b):
    in_maps = _in_maps(template, tomatch, W, b)
    nc = _get_nc()
    res = run_bass_kernel_spmd(nc, in_maps, list(range(N_CORES)))
    return np.stack([res.results[i]["out"] for i in range(N_CORES)])


# revision 12
# speedup vs baseline: 1.8119x; 1.0733x over previous
"""DenseCorr2d full kernel for 8 Trainium2 NeuronCores.

Reference computation (per example b):
  corr[(cm*16+ct), y, x] = sum_{dy,dx} tm_edgepad[cm, y+dy, x+dx] * tp[ct, dy, dx]
  out[co, y, x] = bias[co] + sum_{ci,ky,kx} W[co, ci, ky, kx] * corr_zpad[ci, y+ky-1, x+kx-1]

Sharding: data-parallel over batch; core i computes example i entirely.

Stage A (dense correlation) folds the template taps into the matmul
contraction dim: with y = 8w + j and dx = 4a + de, the contraction rows are
(f = j+dy, de) = 92 partitions, the stationary columns are (ct, j) = 128
(fully dense), and accumulation over a happens in PSUM (4 matmuls per
128-col x 512-spatial tile).  The moving operand baseT[cm, (f,de), w, x'] =
tm_pad[cm, 8w+f, x'+de] is precomputed on host so each tile load is a
contiguous-per-partition DMA.

corr lives in SBUF j-interleaved: corr[ci, (c, j, ws, x)] holds row
y = 8*(ws-1) + j of chunk c (ws=0/17 are the y=-1/128 zero-pad rows, x is
130 wide with zero x-borders).  This makes the PSUM evacuation a full-rate
copy (cast fp32->bf16 into a staging tile) plus ONE contiguous-run DMA per
cm, and keeps every stage-B tap window a contiguous slice within a slab.

Stage B runs the 3x3 'same' merge conv over residue bands (fixed j0, a
window of w): 18 PSUM-accumulating matmuls (9 taps x 2 input-channel
chunks) per band, tap shifts being flat offsets into the slab; bias fused
into the ScalarE PSUM->SBUF copy, which scatters rows back into the
row-major output tile.

All matmuls run in bf16 (inputs are unit-normal; accumulation in fp32
PSUM keeps the relative error ~4e-3, well inside the 2e-2 gate).
"""

from contextlib import ExitStack

import ml_dtypes
import numpy as np

import concourse.bass as bass
import concourse.tile as tile
from concourse import bacc, mybir
from concourse.bass_utils import run_bass_kernel_spmd

F32 = mybir.dt.float32
BF16 = mybir.dt.bfloat16

N_CORES = 8
# Problem shapes (hardcoded per contract).
B, CT, HT, WT = 8, 16, 16, 16
CM, HM, WM = 16, 128, 128
COUT, K = 64, 3
HP = HM + HT - 1  # 143 padded image rows/cols
NF = 23  # f = j + dy range
NP = 4 * NF  # 92 contraction rows (f, de)
XW = 140  # x' range of baseT (x + 4a <= 127+12)
SLAB = 18 * 130  # corr slab (c, j): 18 ws-rows of 130
NCORR = 16 * SLAB  # 2 chunks * 8 j

# stage-B w-window split: 16 w values per j0-residue, psum <= 512 fp32
RWS = [3, 3, 3, 3, 2, 2]
W0S = [0, 3, 6, 9, 12, 14]

_CACHE: dict = {}


def _emit(ctx: ExitStack, tc, nc, btT, sa2, wst, bia, zzb, out):
    const = ctx.enter_context(tc.tile_pool(name="const", bufs=1))
    corrp = ctx.enter_context(tc.tile_pool(name="corrp", bufs=1))

    sa2_sb = const.tile([NP, 4, 128], BF16, name="sa2_sb")
    nc.sync.dma_start(out=sa2_sb[:], in_=sa2.ap())
    w_sb = const.tile([128, 18, COUT], BF16, name="w_sb")
    nc.scalar.dma_start(out=w_sb[:, :9], in_=wst.ap()[:, :9])
    nc.scalar.dma_start(out=w_sb[:, 9:], in_=wst.ap()[:, 9:])
    b_sb = const.tile([COUT, 1], F32, name="b_sb")
    nc.scalar.dma_start(out=b_sb[:], in_=bia.ap())

    corr_sb = corrp.tile([128, NCORR + 2], BF16, name="corr_sb")
    corr_flat = corr_sb[:]
    # slab view: [p, c*8+j, ws*130+x]
    corr_j = corr_sb[:, :NCORR].rearrange("p (s t) -> p s t", s=16)
    # tail slack (zero-pad rows/cols arrive with the per-cm slab shuffle)
    nc.scalar.dma_start(out=corr_sb[:, NCORR:], in_=zzb.ap()[:, :2])

    # ---- Stage A ----
    with (
        tc.tile_pool(name="btp", bufs=2) as btp,
        tc.tile_pool(name="stp0", bufs=2) as stp0,
        tc.tile_pool(name="stp1", bufs=2) as stp1,
        tc.tile_pool(name="psA", bufs=8, space="PSUM") as psA,
    ):
        stps = [stp0, stp1]
        for pr in range(8):
            # one big dma_start per pair: the HW splits an InstDMACopy
            # across SDMA engines at packet granularity, and every
            # dma_start's packets start filling at engine slot 0 — so few
            # big DMAs spread far better than many small ones.
            bt = btp.tile([NP, 2, 16, XW], BF16, name="bt", tag="bt")
            nc.sync.dma_start(out=bt[:], in_=btT.ap()[pr])
            pts = [
                [
                    psA.tile([128, 4, 128], F32, name=f"pA{cm2}_{ybg}", tag="pA")
                    for ybg in range(4)
                ]
                for cm2 in range(2)
            ]
            for a in range(4):
                for cm2 in range(2):
                    for ybg in range(4):
                        nc.tensor.matmul(
                            pts[cm2][ybg][:],
                            sa2_sb[:, a, :],
                            bt[:, cm2, 4 * ybg : 4 * ybg + 4, 4 * a : 4 * a + 128],
                            start=(a == 0),
                            stop=(a == 3),
                        )
            for cm2 in range(2):
                cm = 2 * pr + cm2
                # st covers the full 18-row slab: ws=0/17 zero-pad rows and
                # x-border zeros ride along with the data in one shuffle.
                st = stps[cm2].tile([128, 18, 130], BF16, name="st", tag="st")
                nc.gpsimd.memset(st[:, 0, :], 0.0)
                nc.gpsimd.memset(st[:, 17, :], 0.0)
                nc.gpsimd.memset(st[:, 1:17, 0], 0.0)
                nc.gpsimd.memset(st[:, 1:17, 129], 0.0)
                for ybg in range(4):
                    dst_sl = st[:, 1 + 4 * ybg : 5 + 4 * ybg, 1:129]
                    if ybg % 2 == 0:
                        nc.vector.tensor_copy(dst_sl, pts[cm2][ybg][:])
                    else:
                        nc.scalar.copy(dst_sl, pts[cm2][ybg][:])
                # st[(8ct+j), ws, x] -> corr slabs (c, j) whole
                c, cmh = cm // 8, cm % 8
                eng = nc.sync if cm2 == 0 else nc.scalar
                eng.dma_start(
                    out=corr_j[16 * cmh : 16 * cmh + 16, c * 8 : c * 8 + 8, :],
                    in_=st[:],
                )

    # ---- Stage B ----
    def slab_off(c, j0, ky, w0):
        jj = j0 + ky - 1
        if jj < 0:
            return (c * 8 + 7) * SLAB + w0 * 130
        if jj > 7:
            return c * 8 * SLAB + (w0 + 2) * 130
        return (c * 8 + jj) * SLAB + (w0 + 1) * 130

    with (
        tc.tile_pool(name="psB", bufs=4, space="PSUM") as psB,
        tc.tile_pool(name="outp", bufs=2) as outp,
    ):
        for wg in range(len(RWS)):
            w0, rw = W0S[wg], RWS[wg]
            ot = outp.tile([COUT, 8 * rw, WM], F32, name="ot", tag="ot")
            for j0 in range(8):
                n = rw * 130
                pb = psB.tile([COUT, 3 * 130], F32, name="pb", tag="pb")
                first = True
                for c in range(2):
                    for s in range(9):
                        ky, kx = divmod(s, 3)
                        off = slab_off(c, j0, ky, w0) + kx
                        nc.tensor.matmul(
                            pb[:, :n],
                            w_sb[:, c * 9 + s, :],
                            corr_flat[:, off : off + n],
                            start=first,
                            stop=(c == 1 and s == 8),
                        )
                        first = False
                nc.scalar.activation(
                    ot[:, j0 : j0 + 8 * (rw - 1) + 1 : 8, :],
                    pb[:, :n].rearrange("p (a b) -> p a b", b=130)[:, :, 0:128],
                    mybir.ActivationFunctionType.Identity,
                    bias=b_sb[:, 0:1],
                )
            nc.scalar.dma_start(
                out=out.ap()[:, 8 * w0 : 8 * w0 + 8 * rw, :],
                in_=ot[:],
            )


def _build():
    nc = bacc.Bacc("TRN2", target_bir_lowering=False, debug=False)
    btT = nc.dram_tensor("btT", [8, NP, 2, 16, XW], BF16, kind="ExternalInput")
    sa2 = nc.dram_tensor("sa2", [NP, 4, 128], BF16, kind="ExternalInput")
    wst = nc.dram_tensor("wst", [128, 18, COUT], BF16, kind="ExternalInput")
    bia = nc.dram_tensor("bias", [COUT, 1], F32, kind="ExternalInput")
    zzb = nc.dram_tensor("zzb", [128, 130], BF16, kind="ExternalInput")
    out = nc.dram_tensor("out", [COUT, HM, WM], F32, kind="ExternalOutput")
    with tile.TileContext(nc) as tc, ExitStack() as ctx:
        _emit(ctx, tc, nc, btT, sa2, wst, bia, zzb, out)
    nc.compile()
    return nc


def _get_nc():
    if "nc" not in _CACHE:
        _CACHE["nc"] = _build()
    return _CACHE["nc"]


def _host_prep(template, tomatch, W, b):
    template = np.ascontiguousarray(template, dtype=np.float32)
    tomatch = np.ascontiguousarray(tomatch, dtype=np.float32)
    W = np.ascontiguousarray(W, dtype=np.float32)
    b = np.ascontiguousarray(b, dtype=np.float32)
    bf = ml_dtypes.bfloat16

    tm_pad = np.pad(
        tomatch, ((0, 0), (0, 0), (0, HT - 1), (0, WT - 1)), mode="edge"
    )  # [B, CM, 143, 143]

    # baseT[b, cm, 4f+de, w, x'] = tm_pad[b, cm, 8w+f, x'+de]
    s0, s1, s2, s3 = tm_pad.strides
    bview = np.lib.stride_tricks.as_strided(
        tm_pad,
        shape=(B, CM, NF, 4, 16, XW),
        strides=(s0, s1, s2, s3, 8 * s2, s3),
    )
    # pair-grouped for one-DMA-per-pair loads: [b, pr, p, cm2, w, x']
    btT = (
        np.ascontiguousarray(bview)
        .reshape(B, 8, 2, NP, 16, XW)
        .transpose(0, 1, 3, 2, 4, 5)
        .astype(bf)
    )
    btT = np.ascontiguousarray(btT)

    # sa2[b, 4f+de, a, 8ct+j] = template[b, ct, f-j, 4a+de] for 0<=f-j<16
    sa2 = np.zeros((B, NP, 4, 128), np.float32)
    tview = template.reshape(B, CT, HT, 4, 4)  # [b, ct, dy, a, de]
    for j in range(8):
        for dy in range(HT):
            f = j + dy
            # [b, de, a, ct] slab
            sa2[:, 4 * f : 4 * f + 4, :, j::8] = tview[:, :, dy].transpose(
                0, 3, 2, 1
            )
    sa2 = sa2.astype(bf)

    # wst[k, c*9 + ky*3 + kx, co] = W[co, c*128+k, ky, kx]
    wst = np.ascontiguousarray(
        W.reshape(COUT, 2, 128, K, K).transpose(2, 1, 3, 4, 0).reshape(128, 18, COUT)
    ).astype(bf)
    bias = np.ascontiguousarray(b.reshape(COUT, 1))
    zzb = np.zeros((128, 130), bf)
    return btT, sa2, wst, bias, zzb


def _in_maps(template, tomatch, W, b):
    btT, sa2, wst, bias, zzb = _host_prep(template, tomatch, W, b)
    return [
        {"btT": btT[i], "sa2": sa2[i], "wst": wst, "bias": bias, "zzb": zzb}
        for i in range(N_CORES)
    ]


def kernel(template, tomatch, W, b):
    in_maps = _in_maps(template, tomatch, W, b)
    nc = _get_nc()
    res = run_bass_kernel_spmd(nc, in_maps, list(range(N_CORES)))
    return np.stack([res.results[i]["out"] for i in range(N_CORES)])
